# revision 9
# baseline (speedup 1.0000x reference)
"""DTCWT forward level-1 (dense separable CNN) on 8 Trainium2 NeuronCores.

Strategy
--------
Pure data parallel: the (4, 32, 512, 512) input is 128 independent
512x512 images; each of the 8 cores processes 16 of them.

Per image, both separable filter stages are expressed as TensorE
matmuls against banded 512x512 matrices built on the host from the
filter taps (symmetric padding folded into the band edges):

  stage 1 (contract H):  T[w, h'] = sum_h X[h, w] * B[h, h']
      lhsT = X tile [128(h), 128(w-slice)], rhs = B row-block window.
      Output layout is transposed to [w, h'].
  stage 2 (contract W):  Y[h', w'] = sum_w T[w, h'] * B[w, w']
      Output layout is back to [h', w'].

Only the nonzero band window (<=134 cols) of each B row-block is
streamed per matmul; windows of adjacent k-blocks overlap by a few
columns and accumulate correctly via PSUM's per-element has_written
bit (start=True only on the first matmul into each bank).

q2c's 2x2 decimation is folded into even/odd-column variants of the
band matrices, so the four quads a,b,c,d of each subband are produced
directly by matmuls ([a|d] and [b|c] packed into single PSUM banks).
The butterflies a-d, a+d, b+c, b-c run on VectorE; PSUM evictions on
ScalarE.

Scale folding: q2c's 1/sqrt(2) is folded into the stage-1 highpass
band (T_hi branch -> LoHi, HiHi subbands) and into the stage-2
decimated highpass band used on T_lo (HiLo subbands). LoLo stays
unscaled.
"""

import numpy as np

N = 512          # image H = W
P = 128          # partitions
KB = 4           # 512 / 128 k-blocks
NQ = 256         # decimated size
NCORES = 8
IMGS_PER_CORE = 16


# ---------------------------------------------------------------- host math

def _band(h, n=N):
    """B[j, i] = coefficient of input row j in output i for the 1-D
    cross-correlation with symmetric padding (matches jnp.pad
    mode='symmetric' + conv_general_dilated VALID)."""
    m = len(h)
    p = m // 2
    B = np.zeros((n, n), dtype=np.float64)
    for i in range(n):
        for k in range(m):
            t = i + k - p
            if t < 0:
                t = -t - 1
            if t >= n:
                t = 2 * n - 1 - t
            B[t, i] += float(h[k])
    return B


def _windows(B):
    """Per 128-row block, [lo, hi) range of nonzero columns."""
    wins = []
    for k in range(KB):
        nz = np.nonzero(B[k * P:(k + 1) * P].any(axis=0))[0]
        wins.append((int(nz.min()), int(nz.max()) + 1))
    return wins


def make_consts(h0o, h1o, sim_trim=False):
    """All band-matrix constants (float32) + their nonzero windows.

    sim_trim=True zeroes band entries outside each k-block's own column
    cell so windows never overlap (CoreSim can't model per-element PSUM
    has_written). Numerically wrong at ~6-col strips; used only to
    validate plumbing in simulation against a matching host model."""
    sq = 1.0 / np.sqrt(2.0)
    B0 = _band(h0o)
    B1 = _band(h1o)
    c = {
        "b0": B0,                      # stage1 lowpass + stage2 LoLo
        "b1s": B1 * sq,                # stage1 highpass (scale folded)
        "b0we": B0[:, 0::2],           # stage2 decim for LoHi (s0/s5)
        "b0wo": B0[:, 1::2],
        "b1we": B1[:, 0::2],           # stage2 decim for HiHi (s1/s4)
        "b1wo": B1[:, 1::2],
        "b1swe": B1[:, 0::2] * sq,     # stage2 decim for HiLo (s2/s3)
        "b1swo": B1[:, 1::2] * sq,
    }
    if sim_trim:
        for name, B in c.items():
            cell = B.shape[1] // KB
            for k in range(KB):
                B[k * P:(k + 1) * P, :k * cell] = 0.0
                B[k * P:(k + 1) * P, (k + 1) * cell:] = 0.0
    consts = {k: np.ascontiguousarray(v, dtype=np.float32) for k, v in c.items()}
    wins = {k: _windows(v) for k, v in consts.items()}
    return consts, wins


def reference_host(x_imgs, consts):
    """Numpy oracle mirroring the device dataflow exactly (same band
    matrices, including any sim_trim masking)."""
    b0 = consts["b0"].astype(np.float64)
    b1s = consts["b1s"].astype(np.float64)
    dec = {k: consts[k].astype(np.float64)
           for k in ("b0we", "b0wo", "b1we", "b1wo", "b1swe", "b1swo")}
    n = x_imgs.shape[0]
    yl = np.empty((n, N, N), np.float32)
    yhr = np.empty((n, 6, NQ, NQ), np.float32)
    yhi = np.empty((n, 6, NQ, NQ), np.float32)
    for i in range(n):
        img = x_imgs[i].astype(np.float64)
        t = {"tlo": img.T @ b0, "thi": img.T @ b1s}
        yl[i] = t["tlo"].T @ b0
        for (srcname, bwe, bwo, s1, s2) in SUBBANDS:
            src = t[srcname]
            a = src[:, 0::2].T @ dec[bwe]
            b = src[:, 0::2].T @ dec[bwo]
            cc = src[:, 1::2].T @ dec[bwe]
            d = src[:, 1::2].T @ dec[bwo]
            yhr[i, s1] = a - d
            yhi[i, s1] = b + cc
            yhr[i, s2] = a + d
            yhi[i, s2] = b - cc
    return yl, yhr, yhi


# ---------------------------------------------------------------- device IR

# subband plan: (source tile name, even-B name, odd-B name, s1, s2)
SUBBANDS = (
    ("thi", "b0we", "b0wo", 0, 5),
    ("thi", "b1we", "b1wo", 1, 4),
    ("tlo", "b1swe", "b1swo", 2, 3),
)


def build_nc(n_img, wins):
    """Build the per-core Bass program for n_img 512x512 images."""
    import concourse.bacc as bacc
    import concourse.tile as tile
    from concourse import mybir
    from contextlib import ExitStack

    f32 = mybir.dt.float32
    nc = bacc.Bacc("TRN2", target_bir_lowering=False, debug=False)

    x = nc.dram_tensor("x", (n_img, N, N), f32, kind="ExternalInput")
    dconst = {
        name: nc.dram_tensor(name, (N, NQ if name.endswith(("we", "wo")) else N),
                             f32, kind="ExternalInput")
        for name in ("b0", "b1s", "b0we", "b0wo", "b1we", "b1wo", "b1swe", "b1swo")
    }
    yl = nc.dram_tensor("yl", (n_img, N, N), f32, kind="ExternalOutput")
    yhr = nc.dram_tensor("yhr", (n_img, 6, NQ, NQ), f32, kind="ExternalOutput")
    yhi = nc.dram_tensor("yhi", (n_img, 6, NQ, NQ), f32, kind="ExternalOutput")

    with tile.TileContext(nc) as tc, ExitStack() as ctx:
        const = ctx.enter_context(tc.tile_pool(name="const", bufs=1))
        xpool = ctx.enter_context(tc.tile_pool(name="xp", bufs=3))
        tpool = ctx.enter_context(tc.tile_pool(name="tp", bufs=2))
        stage = ctx.enter_context(tc.tile_pool(name="st", bufs=4))
        abpool = ctx.enter_context(tc.tile_pool(name="ab", bufs=4))
        zpool = ctx.enter_context(tc.tile_pool(name="zp", bufs=12))
        pbig = ctx.enter_context(tc.tile_pool(name="pb", bufs=4, space="PSUM"))
        psub = ctx.enter_context(tc.tile_pool(name="ps", bufs=4, space="PSUM"))

        # constants -> SBUF, laid out [partition, k-block, col]
        csb = {}
        for name, dr in dconst.items():
            ncol = dr.shape[1]
            t = const.tile([P, KB, ncol], f32, tag=f"c_{name}", name=f"c_{name}")
            nc.sync.dma_start(out=t[:], in_=dr[:].rearrange("(k p) n -> p k n", p=P))
            csb[name] = t

        for i in range(n_img):
            # ---- load image: xt[:, k, :] = X rows [128k, 128k+128)
            xt = xpool.tile([P, KB, N], f32, tag="xt")
            nc.sync.dma_start(out=xt[:],
                              in_=x[i].rearrange("(k p) w -> p k w", p=P))

            # ---- stage 1: T[w-block m] = sum_h X[h, w] B[h, h']
            tsb = {"tlo": tpool.tile([P, KB, N], f32, tag="tlo", name="tlo"),
                   "thi": tpool.tile([P, KB, N], f32, tag="thi", name="thi")}
            for m in range(KB):
                p_lo = pbig.tile([P, N], f32, tag="pbig")
                p_hi = pbig.tile([P, N], f32, tag="pbig")
                for k in range(KB):
                    lhsT = xt[:, k, m * P:(m + 1) * P]
                    w0, w1 = wins["b0"][k]
                    nc.tensor.matmul(p_lo[:, w0:w1], lhsT, csb["b0"][:, k, w0:w1],
                                     start=(k == 0), stop=(k == KB - 1))
                    w0, w1 = wins["b1s"][k]
                    nc.tensor.matmul(p_hi[:, w0:w1], lhsT, csb["b1s"][:, k, w0:w1],
                                     start=(k == 0), stop=(k == KB - 1))
                nc.scalar.copy(tsb["tlo"][:, m, :], p_lo[:])
                nc.scalar.copy(tsb["thi"][:, m, :], p_hi[:])

            # ---- stage 2 LoLo: Yl[h'-slice m] = sum_w T_lo[w, h'] B0[w, w']
            ll = stage.tile([P, KB, N], f32, tag="lolo", name="ll")
            for m in range(KB):
                pll = pbig.tile([P, N], f32, tag="pbig")
                for k in range(KB):
                    lhsT = tsb["tlo"][:, k, m * P:(m + 1) * P]
                    w0, w1 = wins["b0"][k]
                    nc.tensor.matmul(pll[:, w0:w1], lhsT, csb["b0"][:, k, w0:w1],
                                     start=(k == 0), stop=(k == KB - 1))
                nc.scalar.copy(ll[:, m, :], pll[:])
            nc.sync.dma_start(out=yl[i].rearrange("(m p) w -> p m w", p=P),
                              in_=ll[:])

            # ---- stage 2 subbands (decimated, quads packed 2-per-bank)
            for (srcname, bwe, bwo, s1, s2) in SUBBANDS:
                src = tsb[srcname]
                z1r = zpool.tile([P, 2, NQ], f32, tag="z", name="z1r")
                z2r = zpool.tile([P, 2, NQ], f32, tag="z", name="z2r")
                z1i = zpool.tile([P, 2, NQ], f32, tag="z", name="z1i")
                z2i = zpool.tile([P, 2, NQ], f32, tag="z", name="z2i")
                for mq in range(2):
                    pa = psub.tile([P, N], f32, tag="psub")  # [a | d]
                    pb = psub.tile([P, N], f32, tag="psub")  # [b | c]
                    for k in range(KB):
                        lhsT_e = src[:, k, NQ * mq: NQ * (mq + 1): 2]
                        lhsT_o = src[:, k, NQ * mq + 1: NQ * (mq + 1): 2]
                        e0, e1 = wins[bwe][k]
                        o0, o1 = wins[bwo][k]
                        nc.tensor.matmul(pa[:, e0:e1], lhsT_e,
                                         csb[bwe][:, k, e0:e1],
                                         start=(k == 0), stop=False)
                        nc.tensor.matmul(pb[:, o0:o1], lhsT_e,
                                         csb[bwo][:, k, o0:o1],
                                         start=(k == 0), stop=False)
                        nc.tensor.matmul(pa[:, NQ + o0:NQ + o1], lhsT_o,
                                         csb[bwo][:, k, o0:o1],
                                         start=False, stop=(k == KB - 1))
                        nc.tensor.matmul(pb[:, NQ + e0:NQ + e1], lhsT_o,
                                         csb[bwe][:, k, e0:e1],
                                         start=False, stop=(k == KB - 1))
                    a_s = abpool.tile([P, NQ], f32, tag="ab")
                    b_s = abpool.tile([P, NQ], f32, tag="ab")
                    nc.scalar.copy(a_s[:], pa[:, 0:NQ])
                    nc.scalar.copy(b_s[:], pb[:, 0:NQ])
                    nc.vector.tensor_sub(z1r[:, mq, :], a_s[:], pa[:, NQ:N])
                    nc.vector.tensor_add(z2r[:, mq, :], a_s[:], pa[:, NQ:N])
                    nc.vector.tensor_add(z1i[:, mq, :], b_s[:], pb[:, NQ:N])
                    nc.vector.tensor_sub(z2i[:, mq, :], b_s[:], pb[:, NQ:N])
                rearr = "(q p) w -> p q w"
                nc.sync.dma_start(out=yhr[i, s1].rearrange(rearr, p=P), in_=z1r[:])
                nc.sync.dma_start(out=yhi[i, s1].rearrange(rearr, p=P), in_=z1i[:])
                nc.sync.dma_start(out=yhr[i, s2].rearrange(rearr, p=P), in_=z2r[:])
                nc.sync.dma_start(out=yhi[i, s2].rearrange(rearr, p=P), in_=z2i[:])

    nc.compile()
    return nc


# ---------------------------------------------------------------- entry

_CACHE = {}


def _get_nc(n_img, wins):
    key = (n_img, tuple(sorted((k, tuple(v)) for k, v in wins.items())))
    if key not in _CACHE:
        _CACHE[key] = build_nc(n_img, wins)
    return _CACHE[key]


def kernel_with_results(input, h0o, h1o, **run_kwargs):
    from concourse.bass_utils import run_bass_kernel_spmd

    x = np.ascontiguousarray(np.asarray(input, dtype=np.float32)
                             .reshape(NCORES * IMGS_PER_CORE, N, N))
    consts, wins = make_consts(np.asarray(h0o), np.asarray(h1o))
    nc = _get_nc(IMGS_PER_CORE, wins)

    in_maps = []
    for c in range(NCORES):
        m = {"x": x[c * IMGS_PER_CORE:(c + 1) * IMGS_PER_CORE]}
        m.update(consts)
        in_maps.append(m)

    bres = run_bass_kernel_spmd(nc, in_maps, core_ids=list(range(NCORES)),
                                **run_kwargs)
    res = bres.results
    yl = np.concatenate([r["yl"] for r in res], axis=0).reshape(4, 32, N, N)
    yhr = np.concatenate([r["yhr"] for r in res], axis=0).reshape(4, 32, 6, NQ, NQ)
    yhi = np.concatenate([r["yhi"] for r in res], axis=0).reshape(4, 32, 6, NQ, NQ)
    return (yl, yhr, yhi), bres


def kernel(input, h0o, h1o):
    out, _ = kernel_with_results(input, h0o, h1o)
    return out
